# revision 1
# baseline (speedup 1.0000x reference)
"""Neural CDE (RK4, 10 steps) Trainium2 Bass/Tile kernel.

Data-parallel over batch: B=1024 split as 128 per core across 8 NeuronCores.
Weights replicated; no collectives.

Per-core math (BS=128 on SBUF partitions):
  z0 = a[:,0] @ W_init + b_init
  per RK4 stage:  dX is one of 21 precomputed vectors (spline derivative at
  the stage's time, which only depends on coeffs, not z):
      hT   = tanh(W1.T-contract zT + b1)           (PE + ACT, h on partitions)
      F    = hT.T @ W2  in 1024-wide chunks        (PE, bf16, f32 PSUM accum)
      k    = segmented-reduce_c(F * rep(dX)) + dX @ b2r.T   (ACT copy, DVE
             mul + reduce, small PE matmul for the b2 term)
  RK4 combine in f32 on DVE. Output out[t] = z_t @ W_out + b_out per step.
"""

import sys
import numpy as np

for _p in ("/opt/trn_rl_repo",):
    if _p not in sys.path:
        sys.path.insert(0, _p)

import ml_dtypes
from contextlib import ExitStack

import concourse.bass as bass
import concourse.bacc as bacc
import concourse.mybir as mybir
import concourse.tile as tile
from concourse.masks import make_identity
from concourse.bass_utils import run_bass_kernel_spmd

B, T, C, H = 1024, 11, 64, 256
NCORES = 8
BS = B // NCORES          # 128
HC = H * C                # 16384
CHUNK = 1024              # F free-dim chunk = 2 matmul windows of 512
NCHUNK = HC // CHUNK      # 16
NW = CHUNK // 512         # windows per chunk

f32 = np.float32
bf16 = ml_dtypes.bfloat16
FP32 = mybir.dt.float32
BF16 = mybir.dt.bfloat16
AO = mybir.AluOpType
AF = mybir.ActivationFunctionType
AX = mybir.AxisListType


def _stage_consts(t_span: np.ndarray):
    """Host-side f32 scalar constants mimicking the reference's fp32 ops."""
    t = np.asarray(t_span, dtype=f32)
    cs = []
    for i in range(T - 1):
        t0 = t[i]
        dt = f32(t[i + 1] - t0)
        tm = f32(t0 + f32(f32(0.5) * dt))
        idx_m = int(np.clip(np.searchsorted(t, tm, side="right") - 1, 0, T - 2))
        fm = f32(tm - t[idx_m])
        cs.append((float(dt), idx_m, float(fm)))
    # final-stage frac for step T-2 (t lands on t_span[-1], idx clamps to T-2)
    fr_last = f32(t[T - 1] - t[T - 2])
    return cs, float(fr_last)


def _build_program(t_span: np.ndarray):
    cs, fr_last = _stage_consts(t_span)

    nc = bacc.Bacc("TRN2", target_bir_lowering=False, debug=False,
                   enable_asserts=False, num_devices=NCORES)

    coeffs_d = nc.dram_tensor("coeffs", [BS, T - 1, 4 * C], FP32, kind="ExternalInput").ap()
    w1_d = nc.dram_tensor("w1", [H, H], BF16, kind="ExternalInput").ap()
    w2_d = nc.dram_tensor("w2", [H, HC], BF16, kind="ExternalInput").ap()
    b1_d = nc.dram_tensor("b1", [H], FP32, kind="ExternalInput").ap()
    b2rt_d = nc.dram_tensor("b2rt", [C, H], BF16, kind="ExternalInput").ap()
    winit_d = nc.dram_tensor("winit", [C, H], BF16, kind="ExternalInput").ap()
    wout_d = nc.dram_tensor("wout", [H, C], FP32, kind="ExternalInput").ap()
    binit_d = nc.dram_tensor("binit", [1, H], FP32, kind="ExternalInput").ap()
    bout_d = nc.dram_tensor("bout", [1, C], FP32, kind="ExternalInput").ap()
    out_d = nc.dram_tensor("out", [BS, T * C], FP32, kind="ExternalOutput").ap()

    with tile.TileContext(nc) as tc, ExitStack() as ctx:
        const = ctx.enter_context(tc.tile_pool(name="const", bufs=1))
        spool = ctx.enter_context(tc.tile_pool(name="stage", bufs=2))
        zpool = ctx.enter_context(tc.tile_pool(name="z", bufs=2))
        kbpool = ctx.enter_context(tc.tile_pool(name="kb", bufs=5))
        fpool = ctx.enter_context(tc.tile_pool(name="fsb", bufs=6))
        gpool = ctx.enter_context(tc.tile_pool(name="gsb", bufs=4))
        pp = ctx.enter_context(tc.tile_pool(name="psmm", bufs=2, space="PSUM"))
        fp = ctx.enter_context(tc.tile_pool(name="psfp", bufs=2, space="PSUM"))

        # ---- resident tensors -------------------------------------------
        coeffs_sb = const.tile([BS, (T - 1) * 4 * C], FP32, tag="coeffs")
        w1_sb = const.tile([128, 2 * H], BF16, tag="w1")
        w2_sb = const.tile([128, 2 * HC], BF16, tag="w2")
        b1_sb = const.tile([128, 2], FP32, tag="b1")
        b2rt_sb = const.tile([C, H], BF16, tag="b2rt")
        winit_sb = const.tile([C, H], BF16, tag="winit")
        wout_sb = const.tile([128, 2 * C], FP32, tag="wout")
        binit_sb = const.tile([1, H], FP32, tag="binit")
        bout_sb = const.tile([1, C], FP32, tag="bout")
        ones1_sb = const.tile([1, 128], FP32, tag="ones1")
        ident = const.tile([128, 128], FP32, tag="ident")
        binit_rep = const.tile([128, H], FP32, tag="binit_rep")
        bout_rep = const.tile([128, C], FP32, tag="bout_rep")
        dxm_sb = const.tile([128, 11 * C], FP32, tag="dxm")      # 10 mids + last-end
        dxT_sb = const.tile([C, 21 * 128], BF16, tag="dxT")
        dxb_sb = const.tile([128, 21 * C], BF16, tag="dxb")
        bc_all = const.tile([128, 21 * H], BF16, tag="bc_all")   # dX @ b2r.T per stage
        out_sb = const.tile([BS, T * C], FP32, tag="out_sb")

        nc.sync.dma_start(out=coeffs_sb[:], in_=coeffs_d.rearrange("p i j -> p (i j)"))
        nc.sync.dma_start(out=w1_sb.rearrange("p (k h) -> p k h", k=2),
                          in_=w1_d.rearrange("(k p) h -> p k h", p=128))
        nc.sync.dma_start(out=w2_sb.rearrange("p (k m) -> p k m", k=2),
                          in_=w2_d.rearrange("(k p) m -> p k m", p=128))
        nc.sync.dma_start(out=b1_sb[:], in_=b1_d.rearrange("(k p) -> p k", p=128))
        nc.sync.dma_start(out=b2rt_sb[:], in_=b2rt_d)
        nc.sync.dma_start(out=winit_sb[:], in_=winit_d)
        nc.sync.dma_start(out=wout_sb.rearrange("p (k c) -> p k c", k=2),
                          in_=wout_d.rearrange("(k p) c -> p k c", p=128))
        nc.sync.dma_start(out=binit_sb[:], in_=binit_d)
        nc.sync.dma_start(out=bout_sb[:], in_=bout_d)

        nc.vector.memset(ones1_sb[:], 1.0)
        make_identity(nc, ident[:])
        identb = const.tile([128, 128], BF16, tag="identb")
        nc.vector.tensor_copy(identb[:], ident[:])

        def cview(i, part):
            """f32 view of coeff column `part` (0=a,1=b,2=2c,3=3d) of interval i."""
            off = i * 4 * C + part * C
            return coeffs_sb[:, off:off + C]

        def dx_f32(s):
            if s < 10:
                return cview(s, 1)
            return dxm_sb[:, (s - 10) * C:(s - 9) * C]

        # ---- dX mid/end vectors (f32) -----------------------------------
        tmp_pool = ctx.enter_context(tc.tile_pool(name="tmp64", bufs=2))
        for i in range(T - 1):
            dt_i, im, fm = cs[i]
            tmp = tmp_pool.tile([128, C], FP32, tag="t64")
            nc.vector.scalar_tensor_tensor(
                out=tmp[:], in0=cview(im, 3), scalar=float(fm), in1=cview(im, 2),
                op0=AO.mult, op1=AO.add)
            nc.vector.scalar_tensor_tensor(
                out=dxm_sb[:, i * C:(i + 1) * C], in0=tmp[:], scalar=float(fm),
                in1=cview(im, 1), op0=AO.mult, op1=AO.add)
        # end-of-grid vector for the very last stage (s == 20)
        tmp = tmp_pool.tile([128, C], FP32, tag="t64")
        nc.vector.scalar_tensor_tensor(
            out=tmp[:], in0=cview(T - 2, 3), scalar=float(fr_last), in1=cview(T - 2, 2),
            op0=AO.mult, op1=AO.add)
        nc.vector.scalar_tensor_tensor(
            out=dxm_sb[:, 10 * C:11 * C], in0=tmp[:], scalar=float(fr_last),
            in1=cview(T - 2, 1), op0=AO.mult, op1=AO.add)

        # ---- dX transposes (bf16) + bf16 copy + bc = dX @ b2r.T ----------
        for s in range(21):
            src = dx_f32(s)
            ps = pp.tile([128, H], FP32, tag="mm")
            nc.tensor.transpose(ps[0:C, 0:128], src, ident[:])
            nc.scalar.copy(dxT_sb[:, s * 128:(s + 1) * 128], ps[0:C, 0:128])
            nc.scalar.copy(dxb_sb[:, s * C:(s + 1) * C], src)
        for s in range(21):
            bc_ps = pp.tile([128, H], FP32, tag="mm")
            nc.tensor.matmul(bc_ps[:], lhsT=dxT_sb[:, s * 128:(s + 1) * 128],
                             rhs=b2rt_sb[:], start=True, stop=True)
            nc.scalar.copy(bc_all[:, s * H:(s + 1) * H], bc_ps[:])

        def bc_sb(s):
            return bc_all[:, s * H:(s + 1) * H]

        # ---- replicated biases -------------------------------------------
        ps = pp.tile([128, H], FP32, tag="mm")
        nc.tensor.matmul(ps[:, 0:H], lhsT=ones1_sb[:], rhs=binit_sb[:], start=True, stop=True)
        nc.scalar.copy(binit_rep[:], ps[:, 0:H])
        ps = pp.tile([128, H], FP32, tag="mm")
        nc.tensor.matmul(ps[:, 0:C], lhsT=ones1_sb[:], rhs=bout_sb[:], start=True, stop=True)
        nc.scalar.copy(bout_rep[:], ps[:, 0:C])

        # ---- z0 ----------------------------------------------------------
        ps = pp.tile([128, H], FP32, tag="mm")
        nc.tensor.transpose(ps[0:C, 0:128], cview(0, 0), ident[:])
        x0T_sb = spool.tile([C, 128], BF16, tag="x0T")
        nc.scalar.copy(x0T_sb[:], ps[0:C, 0:128])
        ps = pp.tile([128, H], FP32, tag="mm")
        nc.tensor.matmul(ps[:, 0:H], lhsT=x0T_sb[:], rhs=winit_sb[:], start=True, stop=True)
        z = zpool.tile([BS, H], FP32, tag="z")
        nc.vector.tensor_tensor(out=z[:], in0=ps[:, 0:H], in1=binit_rep[:], op=AO.add)

        # ---- one RK4 stage ----------------------------------------------
        # Returns (ksum, bc_ps, zb, pre): ksum = segmented-reduced F*dX;
        # bc_ps = dX @ b2r.T (PSUM); zb = zbase + alpha*bc (hoisted off the
        # critical tail); pre = pre_add + bc (for the RK4 combine).
        def gstage(zin, s, alpha=None, zbase=None, pre_add=None, emit_out_t=None):
            # transpose zin -> zT; bf16 zin (k2-k4) uses cheap bf16 transposes
            if zin.dtype == mybir.dt.bfloat16:
                zt_psA = pp.tile([128, 128], BF16, tag="ztb")
                zt_psB = pp.tile([128, 128], BF16, tag="ztb")
                nc.tensor.transpose(zt_psA[:], zin[:, 0:128], identb[:])
                nc.tensor.transpose(zt_psB[:], zin[:, 128:256], identb[:])
            else:
                zt_psA = pp.tile([128, 128], FP32, tag="mm")
                zt_psB = pp.tile([128, 128], FP32, tag="mm")
                nc.tensor.transpose(zt_psA[:], zin[:, 0:128], ident[:])
                nc.tensor.transpose(zt_psB[:], zin[:, 128:256], ident[:])
            zTb = spool.tile([128, H], BF16, tag="zTb")
            nc.scalar.copy(zTb[:, 0:128], zt_psA[:])
            nc.scalar.copy(zTb[:, 128:256], zt_psB[:])

            zb = None
            if alpha is not None:
                zb = zpool.tile([BS, H], FP32, tag="zb")
                nc.vector.scalar_tensor_tensor(out=zb[:], in0=bc_sb(s), scalar=float(alpha),
                                               in1=zbase[:], op0=AO.mult, op1=AO.add)
            pre = None
            if pre_add is not None:
                pre = kbpool.tile([BS, H], FP32, tag="pre")
                nc.vector.tensor_tensor(out=pre[:], in0=pre_add[:], in1=bc_sb(s), op=AO.add)

            if emit_out_t is not None:
                t_idx = emit_out_t
                zTf = spool.tile([128, H], FP32, tag="zTf")
                nc.scalar.copy(zTf[:, 0:128], zt_psA[:])
                nc.scalar.copy(zTf[:, 128:256], zt_psB[:])
                ot_ps = pp.tile([128, H], FP32, tag="mm")
                for kc in range(2):
                    nc.tensor.matmul(ot_ps[:, 0:C], lhsT=zTf[:, kc * 128:(kc + 1) * 128],
                                     rhs=wout_sb[:, kc * C:(kc + 1) * C],
                                     start=(kc == 0), stop=(kc == 1))
                nc.vector.tensor_tensor(out=out_sb[:, t_idx * C:(t_idx + 1) * C],
                                        in0=ot_ps[:, 0:C], in1=bout_rep[:], op=AO.add)

            # hT = tanh(W1.T zT + b1)
            ht_ps = pp.tile([128, H], FP32, tag="mm")
            for hck in range(2):
                for kc in range(2):
                    nc.tensor.matmul(
                        ht_ps[:, hck * 128:(hck + 1) * 128],
                        lhsT=w1_sb[:, kc * H + hck * 128: kc * H + (hck + 1) * 128],
                        rhs=zTb[:, kc * 128:(kc + 1) * 128],
                        start=(kc == 0), stop=(kc == 1))
            hT0 = spool.tile([128, 128], BF16, tag="hT0")
            hT1 = spool.tile([128, 128], BF16, tag="hT1")
            for hck, ht_t in enumerate((hT0, hT1)):
                nc.scalar.activation(ht_t[:],
                                     ht_ps[:, hck * 128:(hck + 1) * 128],
                                     AF.Tanh, bias=b1_sb[:, hck:hck + 1], scale=1.0)
            hT_half = (hT0, hT1)

            # F chunks -> scaled -> segment-reduced
            # ksum in bf16 so the DVE reduce hits 2x packed mode (fp32 out
            # forces 1x); dX multiplicand is a stride-0 broadcast view.
            ksum = kbpool.tile([BS, H], BF16, tag="ksum")
            dxbc = dxb_sb[:, s * C:(s + 1) * C].rearrange("p (r c) -> p r c", r=1)
            # half-width chunks at both ends: the head primes the ACT/DVE
            # pipeline sooner, the tail shortens the end-of-stage drain.
            chunks = ([(0, 512), (512, 512)]
                      + [(1024 * (j + 1), 1024) for j in range(14)]
                      + [(15360, 512), (15872, 512)])
            for off, cw in chunks:
                fps = fp.tile([128, cw], FP32, tag="fp")
                for kc in range(2):
                    for w in range(cw // 512):
                        col = kc * HC + off + w * 512
                        nc.tensor.matmul(fps[:, w * 512:(w + 1) * 512],
                                         lhsT=hT_half[kc][:],
                                         rhs=w2_sb[:, col:col + 512],
                                         start=(kc == 0), stop=(kc == 1),
                                         skip_group_check=True)
                fsb = fpool.tile([128, cw], BF16, tag="fsb")
                nc.scalar.copy(fsb[:], fps[:])
                gsb = gpool.tile([128, cw], BF16, tag="gsb")
                nc.vector.tensor_tensor(out=gsb.rearrange("p (s c) -> p s c", c=C),
                                        in0=fsb.rearrange("p (s c) -> p s c", c=C),
                                        in1=dxbc.broadcast_to([128, cw // C, C]),
                                        op=AO.mult)
                with nc.allow_low_precision(reason="64-wide bf16 segment sums; fp32 internal accum"):
                    nc.vector.tensor_reduce(
                        out=ksum[:, off // C:(off + cw) // C],
                        in_=gsb.rearrange("p (s c) -> p s c", c=C),
                        axis=AX.X, op=AO.add)
            return ksum, zb, pre

        # ---- RK4 time loop ----------------------------------------------
        for i in range(T - 1):
            dt_i, im, fm = cs[i]
            hdt = float(f32(f32(0.5) * f32(dt_i)))
            dt6 = float(f32(f32(dt_i) / f32(6.0)))
            s_m = 10 + i
            s_e = (i + 1) if i < T - 2 else 20

            def kfull(ksum, s):
                kb = kbpool.tile([BS, H], FP32, tag="kb")
                nc.vector.tensor_tensor(out=kb[:], in0=ksum[:], in1=bc_sb(s), op=AO.add)
                return kb

            k1s, zb1, _ = gstage(z, i, alpha=hdt, zbase=z, emit_out_t=i)
            zs = zpool.tile([BS, H], BF16, tag="zs")
            for hh in (slice(0, 128), slice(128, 256)):
                nc.vector.scalar_tensor_tensor(out=zs[:, hh], in0=k1s[:, hh], scalar=hdt,
                                               in1=zb1[:, hh], op0=AO.mult, op1=AO.add)
            kb1 = kfull(k1s, i)

            k2s, zb2, _ = gstage(zs[:], s_m, alpha=hdt, zbase=z)
            zs = zpool.tile([BS, H], BF16, tag="zs")
            for hh in (slice(0, 128), slice(128, 256)):
                nc.vector.scalar_tensor_tensor(out=zs[:, hh], in0=k2s[:, hh], scalar=hdt,
                                               in1=zb2[:, hh], op0=AO.mult, op1=AO.add)
            kb2 = kfull(k2s, s_m)

            k3s, zb3, _ = gstage(zs[:], s_m, alpha=float(dt_i), zbase=z)
            zs = zpool.tile([BS, H], BF16, tag="zs")
            for hh in (slice(0, 128), slice(128, 256)):
                nc.vector.scalar_tensor_tensor(out=zs[:, hh], in0=k3s[:, hh], scalar=float(dt_i),
                                               in1=zb3[:, hh], op0=AO.mult, op1=AO.add)
            kb3 = kfull(k3s, s_m)

            # partial RK4 combine (ready before k4's reduces finish)
            acc = kbpool.tile([BS, H], FP32, tag="acc")
            nc.vector.scalar_tensor_tensor(out=acc[:], in0=kb2[:], scalar=2.0, in1=kb1[:],
                                           op0=AO.mult, op1=AO.add)
            acc2 = kbpool.tile([BS, H], FP32, tag="acc2")
            nc.vector.scalar_tensor_tensor(out=acc2[:], in0=kb3[:], scalar=2.0, in1=acc[:],
                                           op0=AO.mult, op1=AO.add)

            k4s, _, pre = gstage(zs[:], s_e, pre_add=acc2)
            acc3 = kbpool.tile([BS, H], FP32, tag="acc3")
            znew = zpool.tile([BS, H], FP32, tag="z")
            for hh in (slice(0, 128), slice(128, 256)):
                nc.vector.tensor_tensor(out=acc3[:, hh], in0=k4s[:, hh], in1=pre[:, hh], op=AO.add)
                nc.vector.scalar_tensor_tensor(out=znew[:, hh], in0=acc3[:, hh], scalar=dt6,
                                               in1=z[:, hh], op0=AO.mult, op1=AO.add)
            z = znew

        # ---- final out row (t = T-1) ------------------------------------
        zt_psA = pp.tile([128, 128], FP32, tag="mm")
        zt_psB = pp.tile([128, 128], FP32, tag="mm")
        nc.tensor.transpose(zt_psA[:], z[:, 0:128], ident[:])
        nc.tensor.transpose(zt_psB[:], z[:, 128:256], ident[:])
        zTf = spool.tile([128, H], FP32, tag="zTf")
        nc.scalar.copy(zTf[:, 0:128], zt_psA[:])
        nc.scalar.copy(zTf[:, 128:256], zt_psB[:])
        ot_ps = pp.tile([128, H], FP32, tag="mm")
        for kc in range(2):
            nc.tensor.matmul(ot_ps[:, 0:C], lhsT=zTf[:, kc * 128:(kc + 1) * 128],
                             rhs=wout_sb[:, kc * C:(kc + 1) * C],
                             start=(kc == 0), stop=(kc == 1))
        nc.vector.tensor_tensor(out=out_sb[:, (T - 1) * C:T * C],
                                in0=ot_ps[:, 0:C], in1=bout_rep[:], op=AO.add)

        nc.sync.dma_start(out=out_d, in_=out_sb[:])

    nc.compile()
    return nc


_CACHE = {}


def _get_program(t_span: np.ndarray):
    key = np.asarray(t_span, dtype=f32).tobytes()
    if key not in _CACHE:
        _CACHE[key] = _build_program(t_span)
    return _CACHE[key]


def _make_in_maps(inputs):
    coeffs = np.ascontiguousarray(inputs["coeffs"], dtype=f32)
    assert coeffs.shape == (B, T - 1, 4 * C)
    shared = {
        "w1": np.ascontiguousarray(inputs["W1"], dtype=f32).astype(bf16),
        "w2": np.ascontiguousarray(inputs["W2"], dtype=f32).astype(bf16),
        "b1": np.ascontiguousarray(inputs["b1"], dtype=f32),
        "b2rt": np.ascontiguousarray(
            np.asarray(inputs["b2"], dtype=f32).reshape(H, C).T).astype(bf16),
        "winit": np.ascontiguousarray(inputs["W_init"], dtype=f32).astype(bf16),
        "wout": np.ascontiguousarray(inputs["W_out"], dtype=f32),
        "binit": np.ascontiguousarray(inputs["b_init"], dtype=f32).reshape(1, H),
        "bout": np.ascontiguousarray(inputs["b_out"], dtype=f32).reshape(1, C),
    }
    in_maps = []
    for c in range(NCORES):
        m = dict(shared)
        m["coeffs"] = coeffs[c * BS:(c + 1) * BS]
        in_maps.append(m)
    return in_maps


def kernel(coeffs, t_span, W_init, b_init, W1, b1, W2, b2, W_out, b_out):
    nc = _get_program(t_span)
    in_maps = _make_in_maps(dict(coeffs=coeffs, W_init=W_init, b_init=b_init,
                                 W1=W1, b1=b1, W2=W2, b2=b2,
                                 W_out=W_out, b_out=b_out))
    res = run_bass_kernel_spmd(nc, in_maps, list(range(NCORES)))
    shards = [res.results[c]["out"].reshape(BS, T, C) for c in range(NCORES)]
    return np.ascontiguousarray(np.concatenate(shards, axis=0), dtype=f32)


if __name__ == "__main__":
    rng = np.random.default_rng(0)
    demo = dict(
        coeffs=(rng.standard_normal((B, T - 1, 4 * C)) * 0.5).astype(f32),
        t_span=(np.arange(T) * 0.05).astype(f32),
        W_init=(rng.standard_normal((C, H)) / 8).astype(f32),
        b_init=(rng.standard_normal((H,)) * 0.01).astype(f32),
        W1=(rng.standard_normal((H, H)) / 16).astype(f32),
        b1=(rng.standard_normal((H,)) * 0.01).astype(f32),
        W2=(rng.standard_normal((H, HC)) / 16).astype(f32),
        b2=(rng.standard_normal((HC,)) * 0.01).astype(f32),
        W_out=(rng.standard_normal((H, C)) / 16).astype(f32),
        b_out=np.zeros((C,), f32),
    )
    out = kernel(**demo)
    print("out", out.shape, out.dtype, float(np.abs(out).max()))

